# revision 1
# baseline (speedup 1.0000x reference)
"""Trainium2 Bass kernel for CausalSequenceCML.

Math (reference, per step, grid g laid out (B, C, T)):
    mapped  = r * g * (1 - g)
    local   = causal depthwise conv1d(mapped, K, left pad 3)   # per channel
    physics = (1 - eps) * mapped + eps * local
    g'      = (1 - beta) * physics + beta * x0                 # x0 = initial grid

Because r, eps, beta, K are per-channel constants and the conv is linear,
the whole update is affine in a = g*(1-g) = 0.25 - (g-0.5)^2:

    g' = D - C3*sq[t] - C2*sq[t-1] - C1*sq[t-2] - C0*sq[t-3]
    sq = (g - 0.5)^2
    Cj = (1-beta)*eps*r*K[j]             (j = 0, 1, 2)
    C3 = (1-beta)*r*((1-eps) + eps*K[3])
    D  = beta*x0 + 0.25*(C0+C1+C2+C3)

Left-boundary: conv pads mapped (=> a) with zeros, i.e. sq = 0.25 there; the
sq buffer has 3 leading pad columns held at 0.25.

Sharding: C=512 split across 8 cores (64 ch each). Per core the (B=4, 64, T)
block is flattened to 256 rows = 2 SBUF tiles of (128, PAD+T), channels+batch
on partitions, time on the free dim (PAD leading scratch cols, see below).

Engine split, per step per tile (all fp32 — the chaotic map amplifies
rounding ~3e4x over 16 steps: fp32 eps 6e-8 -> 2e-3 max rel out, so 16-bit
or float32r [11 mantissa bits, probed] anywhere fails):
 - ScalarE: sq = Square(g - 0.5) into the padded sq buffer.
 - VectorE: columns [0, PE_SPLIT) via TWO custom-DVE FIR2 instructions
   (hand-authored uOp program FIR2_ANT):
       E  = D - C3*(sq[t]   + (C1/C3)*sq[t-2])
       g' = E - C2*(sq[t-1] + (C0/C2)*sq[t-3])
   Each runs at ~1 elem/cycle and computes 2 taps + merge: the even-shift
   tap reads the instruction's own input stream 2 elements back through the
   DVE datapath's cross-element flops (NEXT_ALU_OUT_B = next block's
   B-flop, which on TRN2 holds that block's result 2 elements earlier —
   measured on HW). The odd taps come from a second instruction whose
   input AP starts one column earlier. The first ~25 outputs of each
   instruction are pipe-fill garbage (the 2-back tap offset only settles
   after the DVE pipeline fills; measured on HW), absorbed by PAD=48
   leading scratch columns on the E/g buffers; the sq lead pads are a
   constant 0.25 so skewed warmup taps read identical values. This
   replaces 4 scalar_tensor_tensor ops (per-element cost 4 -> 2 DVE
   cycles; measured ~25.3us -> ~14.9us per step).
 - TensorE: columns [PE_SPLIT, T) via 5 PSUM-accumulated fp32 matmuls per
   512-col block: W = diag(-Cj) for the 4 taps (time shifts come free via
   the moving operand's AP column offset) plus an identity-diagonal matmul
   that adds D — so PSUM holds g' directly (fp32 matmul is 2-pass LO/HI,
   measured exact to 1e-7).
 - ScalarE copies the finished PSUM blocks to the state buffer.
GPSIMD stays idle: its SBUF port is an exclusive lock shared with DVE's
2-port ops, so concurrent GPSIMD work measured 2.8x slower overall.

Measured variants (For_i chain method, same session): PE_SPLIT 3264 is the
balanced optimum (DVE ~13.8us busy vs PE ~13.9us); DMA/ACT-seeded-PSUM D
(drop the 5th matmul) is neutral-to-worse — a start=False multi-matmul
group drops seeded bank content (single-matmul groups keep it but
stop=True per tap serializes PE); square-split, psum-fused-square and
column-chunked FIR variants all measured neutral or worse.
"""

import copy

import numpy as np

from concourse import bacc, mybir
import concourse.tile as tile
import concourse.dve_ops as _dve_ops_mod
from concourse.bass_utils import run_bass_kernel_spmd
from concourse.dve_spec import Spec, Src0, Src1, C0 as _C0, C1 as _C1, lower as _dve_lower
from concourse.dve_uop import AluInp, AluOp, DelayInp, DveOpSpec

B, T, C = 4, 4096, 512
N_CORES = 8
CPC = C // N_CORES          # channels per core = 64
ROWS = B * CPC              # 256 rows per core
HALVES = ROWS // 128        # 2 SBUF tiles per core
CLAMP = 1e-4
F32 = mybir.dt.float32
PAD = 48                    # scratch lead cols on g/D/E (absorbs DVE pipe-fill skew)
SQPAD = PAD + 1             # sq lead cols (3 math pads + fill-warmup, all 0.25)

# PE offload: TensorE computes all 4 taps + D for columns [PE_SPLIT, T).
PE_SPLIT = 3200
PE_BLOCK = 512
PE_ADD_D = True
# DMA-D: DMA the D block into the PSUM bank each step and skip the 5th
# (identity-diagonal) matmul; the 4 tap matmuls then accumulate on top
# (start=False). Cuts PE cost per column 20 -> 16 cycles.
PE_DMA_D = False
# Emit the PE-region slice of the Square first (separate ACT op) so TensorE
# starts each step's matmuls ~2.7us earlier; the DVE-region slice follows.
SPLIT_SQ = False
# Fuse the PSUM->SBUF copy with the NEXT step's square: for s < steps-1 the
# PE-region square reads g' straight from the PSUM banks (ACT op), so the
# per-step copy only happens on the last step (where g' must materialize
# for the output DMA). Slims the ScalarE chain between steps.
PSUM_SQ = False
# Chunk the DVE region into two column halves, emitted right-half first
# (with the square split to match), so the FIR chain starts ~1.4us earlier
# each step. The right chunk's PAD-wide warmup head overwrites columns the
# left chunk later rewrites correctly (left emitted after right).
CHUNK = False
# Replace the fp32 identity-diag D matmul (4 cycles/col) with TWO fp32r
# identity matmuls over a static hi/lo split of D (1 cycle/col each;
# fp32r keeps 11 mantissa bits, so Dhi+Dlo reproduces D to ~2^-26 —
# rounding-level). PE per-col cost 20 -> 18 cycles. Measured ~0.2us/step
# win at PE_SPLIT=3200 (rebalanced for the cheaper PE column).
PE_R_D = True
# Double-buffer the sq tensor (ping-pong per step): removes the WAR edge
# where the next step's Square must wait for the PE's last tap matmul to
# finish reading sq. Needs PE_R_D (D/E tiles shrink to the regions that
# read them to fit SBUF). Measured neutral (the tile scheduler's per-tile
# pipelining already hides the edge) — keep off.
DBL_SQ = False
# Interleave PE block emission across the two tiles (t0b0, t1b0, t0b1,
# t1b1) instead of per-tile order, spreading psum completions so each
# tile's copies (which gate the next step's square) land earlier.
PE_ILV = False
# Reorder each tile's PE matmuls weight-major (tap k over all blocks
# back-to-back, then Dhi over all blocks, then Dlo) so consecutive
# instructions share a stationary weight and redundant reloads can be
# skipped. Per-bank group order (start first, stop last) is preserved.
PE_WREUSE = False

_compiled = {}


# --- custom DVE op: FIR2_ANT ------------------------------------------------
# out[k] = in1[k] - s0*(in0[k] + s1*in0[k-2])
# Built by hand at the uOp level (the Spec DSL has no delay primitive).
# Verified on HW: exact vs fp32 reference for all k >= 2.

class _HandDveOp:
    """Duck-types concourse.dve_ops.DveOp for _custom_dve + table-gen."""

    def __init__(self, name, spec, uops, rd1):
        self.name = name
        self.spec = spec
        self.uops = uops
        self.rd1 = rd1
        self.subdim = False

    def compile(self, ver):
        assert ver == "v3", "FIR2_ANT authored for TRN2/v3"
        return DveOpSpec(
            name=self.name,
            opcode=_dve_ops_mod.get_dve_sub_opcode(self.name),
            uops=list(self.uops),
            rd1_en=self.rd1,
        )


_fir2_op = None


def _get_fir2():
    global _fir2_op
    if _fir2_op is not None:
        return _fir2_op
    existing = next((o for o in _dve_ops_mod.OPS if o.name == "FIR2_ANT"), None)
    if existing is not None:
        _fir2_op = existing
        return existing
    spec = Spec(
        body=Src1 - (Src0 + Src0 * _C1) * _C0,
        reference=lambda in0, in1, s0, s1, imm2: in1 - (in0 + in0 * s1) * s0,
    )
    uops = _dve_lower(spec, ver="v3")
    assert len(uops) == 1
    u = copy.deepcopy(uops[0])
    chain = {}
    for j, s in enumerate(u.inp):
        if u.inp_enable[j]:
            chain[s.name] = j - 1
    x = AluInp.PREV_DELAY_0 + chain["SRC_0"]
    d = AluInp.PREV_DELAY_0 + chain["SRC_1"]
    c0 = AluInp.PREV_DELAY_0 + chain["CONST_0"]
    c1 = AluInp.PREV_DELAY_0 + chain["CONST_1"]
    used = set(chain.values())
    free = next(l for l in range(6) if l not in used)
    m_lane = AluInp.PREV_DELAY_0 + free

    dp = u.datapath_config
    # blk0: m = s1 * in0[k-2]  (NEXT_ALU_OUT_B = blk1's B-flop = blk1's
    # result 2 elements back)
    dp[0].op = AluOp.MULTIPLY
    dp[0].alu_src0 = AluInp.NEXT_ALU_OUT_B
    dp[0].alu_src1 = c1
    dp[0].alu_out_enable = 1
    # blk1: result = in0[k] (bypass); broadcast into B-flop; carry m on a
    # free delay lane
    dp[1].op = AluOp.BYPASS
    dp[1].alu_src0 = x
    dp[1].alu_src1 = x
    dp[1].alu_out_enable = 1
    dp[1].alu_out_b_enable = 1
    dp[1].delay[free] = DelayInp.PREV_ALU_OUT
    dp[1].delay_enable[free] = 1
    # blk2: u = in0[k] + m
    dp[2].op = AluOp.ADD
    dp[2].alu_src0 = AluInp.PREV_ALU_OUT
    dp[2].alu_src1 = m_lane
    dp[2].alu_out_enable = 1
    # blk3: v = s0 * u
    dp[3].op = AluOp.MULTIPLY
    dp[3].alu_src0 = AluInp.PREV_ALU_OUT
    dp[3].alu_src1 = c0
    dp[3].alu_out_enable = 1
    # blk4: out = in1 - v
    dp[4].op = AluOp.SUBTRACT
    dp[4].alu_src0 = d
    dp[4].alu_src1 = AluInp.PREV_ALU_OUT
    dp[4].alu_out_enable = 1
    for b in range(5, 8):
        dp[b].op = AluOp.BYPASS
        dp[b].alu_src0 = AluInp.PREV_ALU_OUT
        dp[b].alu_src1 = AluInp.PREV_ALU_OUT
        dp[b].alu_out_enable = 1
        dp[b].alu_out_a_enable = 0
        dp[b].alu_out_b_enable = 0

    op = _HandDveOp("FIR2_ANT", spec, (u,), rd1=True)
    _dve_ops_mod.OPS.append(op)
    row = _dve_ops_mod._CUSTOM_DVE_ROW_BASE + len(_dve_ops_mod.OPS) - 1
    assert row < 0x20
    _dve_ops_mod._SUB_OPCODE_FOR_NAME[op.name] = row
    _dve_ops_mod.CUSTOM_DVE_SPECS[op.name] = op.spec
    _fir2_op = op
    return op


# --- kernel build -----------------------------------------------------------

def _build(steps: int, loop_k: int | None = None, pe_split: int | None = None,
           pe_add_d: bool | None = None, pe_dma_d: bool | None = None,
           split_sq: bool | None = None, psum_sq: bool | None = None,
           chunk: bool | None = None, pe_r_d: bool | None = None,
           dbl_sq: bool | None = None, pe_ilv: bool | None = None,
           pe_wreuse: bool | None = None):
    PE_SPLIT = globals()["PE_SPLIT"] if pe_split is None else pe_split
    PE_ADD_D = globals()["PE_ADD_D"] if pe_add_d is None else pe_add_d
    PE_DMA_D = globals()["PE_DMA_D"] if pe_dma_d is None else pe_dma_d
    SPLIT_SQ = globals()["SPLIT_SQ"] if split_sq is None else split_sq
    PSUM_SQ = globals()["PSUM_SQ"] if psum_sq is None else psum_sq
    CHUNK = globals()["CHUNK"] if chunk is None else chunk
    PE_R_D = globals()["PE_R_D"] if pe_r_d is None else pe_r_d
    DBL_SQ = globals()["DBL_SQ"] if dbl_sq is None else dbl_sq
    PE_ILV = globals()["PE_ILV"] if pe_ilv is None else pe_ilv
    PE_WREUSE = globals()["PE_WREUSE"] if pe_wreuse is None else pe_wreuse
    if PE_WREUSE and not (PE_ADD_D and PE_R_D and not PE_DMA_D):
        PE_WREUSE = False
    pe_on_pre = PE_SPLIT < T
    if DBL_SQ and not (pe_on_pre and PE_R_D and PE_ADD_D):
        DBL_SQ = False
    if PSUM_SQ:
        assert PE_ADD_D or PE_DMA_D  # psum must hold g' directly
    if PE_DMA_D:
        PE_ADD_D = False
    pe_on = PE_SPLIT < T
    fir2 = _get_fir2()
    nc = bacc.Bacc("TRN2", target_bir_lowering=False, debug=False)

    x = nc.dram_tensor("x", [ROWS, T], F32, kind="ExternalInput").ap()
    coef = nc.dram_tensor("coef", [ROWS, 6], F32, kind="ExternalInput").ap()
    out = nc.dram_tensor("out", [ROWS, T], F32, kind="ExternalOutput").ap()
    if pe_on:
        wcols = 640 if PE_ADD_D else 512
        wdiag = nc.dram_tensor("wdiag", [ROWS, wcols], F32,
                               kind="ExternalInput").ap()
        wdiag_h = wdiag.rearrange("(h p) c -> h p c", p=128)

    x_h = x.rearrange("(h p) t -> h p t", p=128)
    out_h = out.rearrange("(h p) t -> h p t", p=128)
    coef_h = coef.rearrange("(h p) c -> h p c", p=128)

    mult = mybir.AluOpType.mult
    add = mybir.AluOpType.add

    with tile.TileContext(nc) as tc:
        with tc.tile_pool(name="state", bufs=1) as pool, \
             tc.tile_pool(name="psum", bufs=8, space="PSUM") as pspool:
            neg_half = pool.tile([128, 1], F32, tag="neg_half", name="neg_half")
            nc.vector.memset(neg_half[:], -0.5)
            dve_end0 = PE_SPLIT if pe_on_pre else T
            dw = PAD + (dve_end0 if DBL_SQ else T)  # D/E tile width
            gA, gB, D, E, cf = [], [], [], [], []
            sqq = [[], []]
            for h in range(HALVES):
                gA.append(pool.tile([128, PAD + T], F32, tag=f"gA{h}", name=f"gA{h}"))
                gB.append(pool.tile([128, PAD + T], F32, tag=f"gB{h}", name=f"gB{h}"))
                sqq[0].append(pool.tile([128, SQPAD + T], F32, tag=f"sq{h}", name=f"sq{h}"))
                if DBL_SQ:
                    sqq[1].append(pool.tile([128, SQPAD + T], F32,
                                            tag=f"sqb{h}", name=f"sqb{h}"))
                D.append(pool.tile([128, dw], F32, tag=f"D{h}", name=f"D{h}"))
                E.append(pool.tile([128, dw], F32, tag=f"E{h}", name=f"E{h}"))
                cf.append(pool.tile([128, 6], F32, tag=f"cf{h}", name=f"cf{h}"))
            if not DBL_SQ:
                sqq[1] = sqq[0]
            sq = sqq[0]

            wd = []
            if pe_on:
                for h in range(HALVES):
                    wd.append(pool.tile([128, wcols], F32, tag=f"wd{h}",
                                        name=f"wd{h}"))
                    nc.sync.dma_start(out=wd[h][:], in_=wdiag_h[h])
            Dhi, Dlo, wident_r = [], [], None
            pe_w = T - PE_SPLIT if pe_on else 0
            if pe_on and PE_R_D:
                F32R = mybir.dt.float32r
                # engine-rounded fp32r identity weight (values 0/1, exact)
                wident_r = pool.tile([128, 128], F32R, tag="wir", name="wir")
                nc.scalar.copy(wident_r[:], wd[0][:, 512:640])
                for h in range(HALVES):
                    # PE-region slice only (SBUF budget)
                    Dhi.append(pool.tile([128, pe_w], F32R, tag=f"Dh{h}",
                                         name=f"Dh{h}"))
                    Dlo.append(pool.tile([128, pe_w], F32R, tag=f"Dl{h}",
                                         name=f"Dl{h}"))
            for h in range(HALVES):
                nc.sync.dma_start(out=cf[h][:], in_=coef_h[h])
                nc.sync.dma_start(out=gA[h][:, PAD:PAD + T], in_=x_h[h])
                # pad columns stay at a^2-of-zero = 0.25 forever
                nc.vector.memset(sqq[0][h][:, 0:SQPAD], 0.25)
                if DBL_SQ:
                    nc.vector.memset(sqq[1][h][:, 0:SQPAD], 0.25)
                # scratch lead cols: keep finite (warmup garbage lands here)
                nc.vector.memset(gA[h][:, 0:PAD], 0.0)
                nc.vector.memset(gB[h][:, 0:PAD], 0.0)
                nc.vector.memset(D[h][:, 0:PAD], 0.0)
                # D = beta * x0 + dconst (only the width the kernel reads)
                nc.vector.tensor_scalar(
                    D[h][:, PAD:dw], gA[h][:, PAD:PAD + (dw - PAD)],
                    cf[h][:, 4:5], cf[h][:, 5:6], mult, add,
                )
                if pe_on and PE_R_D:
                    # static hi/lo split of the PE-region D slice (Dhi
                    # rounds on write; Dlo = exact fp32 residual rounded to
                    # fp32r — combined error ~2^-26 relative). When D is
                    # shrunk (DBL_SQ) the PE-region D is staged through E
                    # as scratch.
                    pe_w_ = T - PE_SPLIT
                    if DBL_SQ:
                        dtmp = E[h][:, 0:pe_w_]
                        nc.vector.tensor_scalar(
                            dtmp, gA[h][:, PAD + PE_SPLIT:PAD + T],
                            cf[h][:, 4:5], cf[h][:, 5:6], mult, add,
                        )
                    else:
                        dtmp = D[h][:, PAD + PE_SPLIT:PAD + T]
                    nc.scalar.copy(Dhi[h][:], dtmp)
                    nc.vector.tensor_tensor(
                        Dlo[h][:], dtmp, Dhi[h][:].bitcast(F32),
                        mybir.AluOpType.subtract,
                    )

            dve_end = PE_SPLIT if pe_on else T
            pe_blocks = []
            c = PE_SPLIT
            while c < T:
                n = min(PE_BLOCK, T - c)
                pe_blocks.append((c, n))
                c += n

            def emit_steps():
                prev_psums = []
                for s in range(steps):
                    cur, nxt = (gA, gB) if s % 2 == 0 else (gB, gA)
                    sq = sqq[s % 2]
                    if CHUNK:
                        m = (dve_end // 2) & ~63
                        lo = max(m - PAD - 2, 0)
                        for h in range(HALVES):
                            nc.scalar.activation(
                                sq[h][:, SQPAD + lo:SQPAD + T],
                                cur[h][:, PAD + lo:PAD + T],
                                mybir.ActivationFunctionType.Square,
                                bias=neg_half[:],
                            )
                        for h in range(HALVES):
                            nc.scalar.activation(
                                sq[h][:, SQPAD:SQPAD + lo],
                                cur[h][:, PAD:PAD + lo],
                                mybir.ActivationFunctionType.Square,
                                bias=neg_half[:],
                            )
                    elif pe_on and PSUM_SQ:
                        # DVE region square from cur; PE region square
                        # straight from last step's PSUM banks (or from cur
                        # on the first step)
                        for h in range(HALVES):
                            end = T if s == 0 else dve_end
                            nc.scalar.activation(
                                sq[h][:, SQPAD:SQPAD + end],
                                cur[h][:, PAD:PAD + end],
                                mybir.ActivationFunctionType.Square,
                                bias=neg_half[:],
                            )
                        for (h, c0, n, ps) in prev_psums:
                            nc.scalar.activation(
                                sq[h][:, SQPAD + c0:SQPAD + c0 + n],
                                ps[:, :n],
                                mybir.ActivationFunctionType.Square,
                                bias=neg_half[:],
                            )
                    elif pe_on and SPLIT_SQ:
                        lo = dve_end - 3  # PE taps read sq cols [lo, T)
                        for h in range(HALVES):
                            nc.scalar.activation(
                                sq[h][:, SQPAD + lo:SQPAD + T],
                                cur[h][:, PAD + lo:PAD + T],
                                mybir.ActivationFunctionType.Square,
                                bias=neg_half[:],
                            )
                        for h in range(HALVES):
                            nc.scalar.activation(
                                sq[h][:, SQPAD:SQPAD + lo],
                                cur[h][:, PAD:PAD + lo],
                                mybir.ActivationFunctionType.Square,
                                bias=neg_half[:],
                            )
                    else:
                        for h in range(HALVES):
                            nc.scalar.activation(
                                sq[h][:, SQPAD:SQPAD + T], cur[h][:, PAD:PAD + T],
                                mybir.ActivationFunctionType.Square,
                                bias=neg_half[:],
                            )
                    # PE region: psum accumulates -Cj taps (+D), ScalarE
                    # copies g' out
                    step_psums = []
                    if pe_on:
                        n_mm = 5 if PE_ADD_D else 4
                        if PE_WREUSE:
                            # weight-major emission per tile: allocate all
                            # of this tile's banks, then stream each
                            # stationary weight over every block before
                            # switching weights
                            for h in range(HALVES):
                                pss = []
                                for (c0, n) in pe_blocks:
                                    ps = pspool.tile(
                                        [128, PE_BLOCK], F32, tag="ps",
                                        name=f"ps{s}_{h}_{c0}")
                                    pss.append((c0, n, ps))
                                    step_psums.append((h, c0, n, ps))
                                for k in range(4):
                                    off = SQPAD - k
                                    for (c0, n, ps) in pss:
                                        nc.tensor.matmul(
                                            ps[:, :n],
                                            wd[h][:, k * 128:(k + 1) * 128],
                                            sq[h][:, off + c0:off + c0 + n],
                                            start=(k == 0), stop=False,
                                        )
                                for (c0, n, ps) in pss:
                                    rc = c0 - PE_SPLIT
                                    nc.tensor.matmul(
                                        ps[:, :n], wident_r[:],
                                        Dhi[h][:, rc:rc + n],
                                        start=False, stop=False,
                                    )
                                for (c0, n, ps) in pss:
                                    rc = c0 - PE_SPLIT
                                    nc.tensor.matmul(
                                        ps[:, :n], wident_r[:],
                                        Dlo[h][:, rc:rc + n],
                                        start=False, stop=True,
                                    )
                            hc = []
                        elif PE_ILV:
                            hc = [(h, b) for b in pe_blocks
                                  for h in range(HALVES)]
                        else:
                            hc = [(h, b) for h in range(HALVES)
                                  for b in pe_blocks]
                        for (h, (c0, n)) in hc:
                            ps = pspool.tile([128, PE_BLOCK], F32, tag="ps",
                                         name=f"ps{s}_{h}_{c0}")
                            if PE_DMA_D:
                                # ACT seeds the bank with D; taps then
                                # accumulate on top (start=False)
                                nc.scalar.copy(
                                    ps[:, :n],
                                    D[h][:, PAD + c0:PAD + c0 + n],
                                )
                            for k in range(4):
                                off = SQPAD - k
                                # DMA_D: each tap is its own
                                # start=False/stop=True group — a
                                # multi-member start=False group
                                # drops the seeded bank content
                                # (measured on HW)
                                nc.tensor.matmul(
                                    ps[:, :n],
                                    wd[h][:, k * 128:(k + 1) * 128],
                                    sq[h][:, off + c0:off + c0 + n],
                                    start=(k == 0 and not PE_DMA_D),
                                    stop=(PE_DMA_D
                                          or (k == n_mm - 1 and not (PE_ADD_D and PE_R_D))),
                                    skip_group_check=PE_DMA_D,
                                )
                            if PE_ADD_D and PE_R_D:
                                rc = c0 - PE_SPLIT
                                nc.tensor.matmul(
                                    ps[:, :n], wident_r[:],
                                    Dhi[h][:, rc:rc + n],
                                    start=False, stop=False,
                                )
                                nc.tensor.matmul(
                                    ps[:, :n], wident_r[:],
                                    Dlo[h][:, rc:rc + n],
                                    start=False, stop=True,
                                )
                            elif PE_ADD_D:
                                nc.tensor.matmul(
                                    ps[:, :n], wd[h][:, 512:640],
                                    D[h][:, PAD + c0:PAD + c0 + n],
                                    start=False, stop=True,
                                )
                            step_psums.append((h, c0, n, ps))
                    # DVE region: two FIR2 ops per tile.
                    # Output position j <-> real col t = j - PAD; the first
                    # ~25 outputs are pipe-fill garbage (tap offset settles
                    # to exactly 2-back only after the fill phase), absorbed
                    # by PAD scratch lead cols; lead pads are constant 0.25
                    # so skewed warm-up taps read identical values.
                    if CHUNK:
                        m = (dve_end // 2) & ~63
                        ranges = [(m, dve_end), (0, m)]  # right first
                    else:
                        ranges = [(0, dve_end)]
                    for (a, b) in ranges:
                        # ops over output cols [a, b): out AP starts at col
                        # a with a PAD warmup head landing in [a, a+PAD) —
                        # scratch when a=0, else overwritten by the later
                        # left chunk.
                        wlen = PAD + (b - a)
                        for h in range(HALVES):
                            # E = D - C3*(sq[t] + (C1/C3)*sq[t-2])
                            nc.vector._custom_dve(
                                fir2, out=E[h][:, a:a + wlen],
                                in0=sq[h][:, a + 1:a + 1 + wlen],
                                in1=D[h][:, a:a + wlen],
                                s0=cf[h][:, 0:1], s1=cf[h][:, 1:2],
                            )
                            # g' = E - C2*(sq[t-1] + (C0/C2)*sq[t-3])
                            nc.vector._custom_dve(
                                fir2, out=nxt[h][:, a:a + wlen],
                                in0=sq[h][:, a:a + wlen],
                                in1=E[h][:, a:a + wlen],
                                s0=cf[h][:, 2:3], s1=cf[h][:, 3:4],
                            )
                    for (h, c0, n, ps) in step_psums:
                        if PSUM_SQ and s < steps - 1:
                            continue  # next step squares from psum directly
                        if PE_ADD_D or PE_DMA_D:
                            nc.scalar.copy(nxt[h][:, PAD + c0:PAD + c0 + n],
                                           ps[:, :n])
                        else:
                            nc.vector.scalar_tensor_tensor(
                                nxt[h][:, PAD + c0:PAD + c0 + n], ps[:, :n],
                                -1.0, D[h][:, PAD + c0:PAD + c0 + n],
                                mult, add,
                            )
                    prev_psums = step_psums

            if loop_k is not None:
                with tc.For_i(0, loop_k):
                    emit_steps()
            else:
                emit_steps()

            fin = gA if steps % 2 == 0 else gB
            for h in range(HALVES):
                nc.vector.tensor_scalar(
                    fin[h][:, PAD:PAD + T], fin[h][:, PAD:PAD + T],
                    CLAMP, 1.0 - CLAMP,
                    mybir.AluOpType.max, mybir.AluOpType.min,
                )
                nc.sync.dma_start(out=out_h[h], in_=fin[h][:, PAD:PAD + T])

    nc.compile()
    return nc


def get_nc(steps: int):
    if steps not in _compiled:
        _compiled[steps] = _build(steps)
    return _compiled[steps]


def _host_prep(drive, r, eps, beta, K_causal):
    """Per-core input maps: x (256, T), coef (256, 6), wdiag (256, 640)."""
    drive = np.asarray(drive, np.float32)
    r = np.asarray(r, np.float32)
    eps = np.asarray(eps, np.float32)
    beta = np.asarray(beta, np.float32)
    K = np.asarray(K_causal, np.float32)[:, 0, :]  # (C, 4)

    one_m_b = 1.0 - beta
    C0 = one_m_b * eps * r * K[:, 0]
    C1 = one_m_b * eps * r * K[:, 1]
    C2 = one_m_b * eps * r * K[:, 2]
    C3 = one_m_b * r * ((1.0 - eps) + eps * K[:, 3])
    dconst = 0.25 * (C0 + C1 + C2 + C3)

    pe_on = PE_SPLIT < T
    in_maps = []
    idx = np.arange(128)
    for i in range(N_CORES):
        sl = slice(i * CPC, (i + 1) * CPC)
        xs = np.ascontiguousarray(
            drive[:, :, sl].transpose(0, 2, 1).reshape(ROWS, T), np.float32
        )
        cs = np.stack(
            [np.tile(C3[sl], B), np.tile(C1[sl] / C3[sl], B),
             np.tile(C2[sl], B), np.tile(C0[sl] / C2[sl], B),
             np.tile(beta[sl], B), np.tile(dconst[sl], B)],
            axis=1,
        ).astype(np.float32)
        m = {"x": xs, "coef": np.ascontiguousarray(cs)}
        if pe_on:
            sign = -1.0 if (PE_ADD_D or PE_DMA_D) else 1.0
            blocks = [sign * C3, sign * C2, sign * C1, sign * C0]
            if PE_ADD_D:
                blocks.append(np.ones(C, np.float32))
            wdg = np.zeros((ROWS, 128 * len(blocks)), np.float32)
            for k, arr in enumerate(blocks):
                rows = np.tile(np.asarray(arr, np.float32)[sl], B)  # (ROWS,)
                for h in range(HALVES):
                    wdg[h * 128 + idx, k * 128 + idx] = rows[h * 128 + idx]
            m["wdiag"] = wdg
        in_maps.append(m)
    return in_maps


def kernel(drive, r, eps, beta, K_causal, steps):
    steps = int(steps)
    nc = get_nc(steps)
    in_maps = _host_prep(drive, r, eps, beta, K_causal)
    res = run_bass_kernel_spmd(nc, in_maps, list(range(N_CORES)))
    parts = [
        res.results[i]["out"].reshape(B, CPC, T).transpose(0, 2, 1)
        for i in range(N_CORES)
    ]
    return np.ascontiguousarray(np.concatenate(parts, axis=2), np.float32)



# revision 11
# speedup vs baseline: 1.9725x; 1.9725x over previous
"""Trainium2 Bass kernel for CausalSequenceCML — zA-factored FIR + 2x DVE.

Math (per step, grid laid out (B, C, T), sq[t] = (g[t]-0.5)^2):
    g'[t] = D[t] - C3*sq[t] - C2*sq[t-1] - C1*sq[t-2] - C0*sq[t-3]
    D     = beta*x0 + 0.25*(C0+C1+C2+C3)        (constant across steps)
    left boundary: sq[t<0] == 0.25 (zero-padded `mapped` in the reference)

Key factorization (per channel, host-solved cubic):
    P[t] = sum_j C_j sq[t-j] = alpha*zA[t] + beta_*zA[t-1]
    zA[t] = sq[t] + mu*sq[t-1] + nu*sq[t-2]
with alpha = C3, and (mu, nu, beta_) a real root of the matching system.
This splits the 4-tap FIR into:
  - opA: zA from sq — SINGLE-stream => runs in DVE 2x_2p perf mode at
    2 elem/cycle (custom uop program, both ports on one AP).
  - opB: sq'[t] = ((D-0.5)[t] - (boa*(alpha*zA[t-1]) + alpha*zA[t]))^2 —
    2-stream 1x op with the SQUARE FUSED (ScalarE's per-step square over
    the DVE region disappears). Final step: variant without square, +0.5.

2x_2p mechanics (probed on HW): the engine splits the (flattened) out/in
stream in half; port0/SRC_0 computes the first half (-> write0_lo),
port1/SRC_1 the second (-> write1_lo). Programs duplicate the block chain
per half. 1-element delays come from chain<-CURR_ALU_OUT injection
(measured age-1 in element units at both 1x and 2x); they carry across
the half boundary seamlessly EXCEPT the first 2 outputs of port1. Fix:
the out/in APs are folded as [128, 2, W] rows overlapping by 2 columns —
port0's tail overwrites port1's glitched head (port0 writes those
columns thousands of cycles later).

Pad maintenance: sq pads (cols t<0) must stay 0.25 every step, but opB
rewrites them; the pad update is itself chaotic (errors x~3.9/step). The
D'-buffer pad columns are set on host to fl(P_pad) - 0.5 replicating the
device's exact fp32 rounding sequence, making 0.25 an EXACT fixed point.
Warmup/boundary junk erodes the clean pad region by 3 cols/step from the
left; PADC=64 lead cols absorb 16 steps of erosion.

PE (TensorE) handles columns [x_s, T): 4 diag-weight tap matmuls + 2
fp32r identity matmuls adding D = Dhi+Dlo (baseline PE_R_D scheme).
ScalarE squares the PE region directly from PSUM (bias -0.5). The last
R_STEPS steps run PE taps in fp32r (weights+moving bitcast; 4+2 -> 1+1
cyc/col per tap pair) with a larger PE share X_LATE; rounding error
2^-12/step amplified ~1.9^k stays within budget for R_STEPS<=6
(simulated: rel max 4.2e-3 at R=4, 7.9e-3 at R=6; gate 2e-2).
"""

import copy

import numpy as np

from concourse import bacc, mybir
import concourse.tile as tile
import concourse.dve_ops as _dve_ops_mod
from concourse.ap import AP
from concourse.bass_utils import run_bass_kernel_spmd
from concourse.dve_spec import Spec, Src0, Src1, C0 as _SC0, C1 as _SC1
from concourse.dve_uop import (
    AluInp, AluOp, DelayInp, DveOpSpec, InpSel, OutPath, OutSel, Trigger,
    UopConfig,
)

B, T, C = 4, 4096, 512
N_CORES = 8
CPC = C // N_CORES          # channels per core = 64
ROWS = B * CPC              # 256 rows per core
HALVES = ROWS // 128        # 2 SBUF tiles per core
CLAMP = 1e-4
F32 = mybir.dt.float32
F32R = mybir.dt.float32r

PADC = 64                   # lead pad cols (erosion: ~3/step + warmup)
STEPS = 16
X_EARLY = 3264              # DVE/PE split, fp32-PE steps
X_LATE = 2304               # DVE/PE split, fp32r-PE steps
R_STEPS = 6                 # trailing steps with fp32r PE taps
PE_BLOCK = 512
MAXX = 3392                 # allocation bound for zA/D' widths (x_early <= MAXX)
XLMIN = 2304                # allocation bound for fp32r-side tiles (x_late >= XLMIN)

_compiled = {}


# --- custom DVE uop programs ------------------------------------------------

def _mkuop(rd1: bool) -> UopConfig:
    u = UopConfig()
    u.trigger = (Trigger.SRC_TENSOR_DONE, Trigger.NONE, Trigger.NONE)
    u.require_inp0 = 1
    u.require_inp1 = 1 if rd1 else 0
    return u


def _marker1x(mul: bool) -> UopConfig:
    # regular/2x_1p slots (never selected for these call sites; fp32-only)
    u = _mkuop(False)
    u.enable_input(InpSel.SRC_0, 0)
    u.datapath_config[0].enable_alu(
        AluOp.MULTIPLY if mul else AluOp.ADD,
        AluInp.PREV_ALU_OUT, AluInp.PREV_ALU_OUT)
    for k in range(1, 8):
        u.datapath_config[k].pass_through_alu()
    u.enable_output(OutSel.ALU_OUT, OutPath.WR0_LO)
    return u


def _opa_2x2p() -> UopConfig:
    """zA[t] = ((S[t] + m1[t-1]) + m2[t-1]); m1 = s0*S, m2 = s1*m1[t-1].
    s0 = mu, s1 = nu/mu. Dual per-half chains (SRC_0 -> write0 via chain0,
    SRC_1 -> write1 via ALU spine)."""
    u = _mkuop(True)      # cf requires_src1: port1 active in 2p mode
    u.enable_input(InpSel.SRC_0, 1)    # ch0
    u.enable_input(InpSel.SRC_1, 2)    # ch1
    u.enable_input(InpSel.CONST_0, 3)  # ch2 = mu
    u.enable_input(InpSel.CONST_1, 4)  # ch3 = nu/mu
    dp = u.datapath_config
    dp[0].enable_alu(AluOp.MULTIPLY, AluInp.PREV_DELAY_2, AluInp.PREV_DELAY_0)
    dp[0].enable_delay_from_src(DelayInp.CURR_ALU_OUT, 4)   # m1A age1
    dp[0].pass_through_delay(0, 1, 2, 3)
    dp[1].enable_alu(AluOp.MULTIPLY, AluInp.PREV_DELAY_3, AluInp.PREV_DELAY_4)
    dp[1].enable_delay_from_src(DelayInp.CURR_ALU_OUT, 5)   # m2A age1
    dp[1].pass_through_delay(0, 1, 2, 3, 4)
    dp[2].enable_alu(AluOp.ADD, AluInp.PREV_DELAY_0, AluInp.PREV_DELAY_4)
    dp[2].pass_through_delay(1, 2, 3, 5)
    dp[3].enable_alu(AluOp.ADD, AluInp.PREV_ALU_OUT, AluInp.PREV_DELAY_5)
    dp[3].pass_through_delay(1, 2, 3)
    dp[4].enable_alu(AluOp.MULTIPLY, AluInp.PREV_DELAY_2, AluInp.PREV_DELAY_1)
    dp[4].enable_delay_from_src(DelayInp.CURR_ALU_OUT, 4)   # m1B age1
    dp[4].enable_delay_from_src(DelayInp.PREV_ALU_OUT, 0)   # zA-A carry
    dp[4].pass_through_delay(1, 3)
    dp[5].enable_alu(AluOp.MULTIPLY, AluInp.PREV_DELAY_3, AluInp.PREV_DELAY_4)
    dp[5].enable_delay_from_src(DelayInp.CURR_ALU_OUT, 5)   # m2B age1
    dp[5].pass_through_delay(0, 1, 4)
    dp[6].enable_alu(AluOp.ADD, AluInp.PREV_DELAY_1, AluInp.PREV_DELAY_4)
    dp[6].pass_through_delay(0, 5)
    dp[7].enable_alu(AluOp.ADD, AluInp.PREV_ALU_OUT, AluInp.PREV_DELAY_5)
    dp[7].pass_through_delay(0)
    u.enable_output(OutSel.DELAY_0, OutPath.WR0_LO)
    u.enable_output(OutSel.ALU_OUT, OutPath.WR1_LO)
    return u


def _opb_1x(square: bool) -> UopConfig:
    """m = s0*in0[t]; w = s1*m[t-1]; p = w + m; d = in1[t] - p;
    out = d*d (square) or d + imm2 (final)."""
    u = _mkuop(True)
    u.enable_input(InpSel.SRC_0, 1)    # ch0 = zA
    u.enable_input(InpSel.SRC_1, 2)    # ch1 = D'
    u.enable_input(InpSel.CONST_0, 4)  # ch3 = alpha
    u.enable_input(InpSel.CONST_1, 5)  # ch4 = beta_/alpha
    if not square:
        u.enable_input(InpSel.CONST_2, 6)  # ch5 = imm2 (+0.5)
    dp = u.datapath_config
    dp[0].enable_alu(AluOp.MULTIPLY, AluInp.PREV_DELAY_0, AluInp.PREV_DELAY_3)
    dp[0].enable_delay_from_src(DelayInp.CURR_ALU_OUT, 0)   # m age1
    dp[0].pass_through_delay(1, 4, *(() if square else (5,)))
    dp[1].enable_alu(AluOp.MULTIPLY, AluInp.PREV_DELAY_0, AluInp.PREV_DELAY_4)
    dp[1].enable_delay_from_src(DelayInp.PREV_ALU_OUT, 0)   # m age0
    dp[1].pass_through_delay(1, *(() if square else (5,)))
    dp[2].enable_alu(AluOp.ADD, AluInp.PREV_ALU_OUT, AluInp.PREV_DELAY_0)
    dp[2].pass_through_delay(1, *(() if square else (5,)))
    dp[3].enable_alu(AluOp.SUBTRACT, AluInp.PREV_DELAY_1, AluInp.PREV_ALU_OUT)
    if not square:
        dp[3].pass_through_delay(5)
    if square:
        dp[4].enable_alu(AluOp.MULTIPLY, AluInp.PREV_ALU_OUT,
                         AluInp.PREV_ALU_OUT)
    else:
        dp[4].enable_alu(AluOp.ADD, AluInp.PREV_ALU_OUT, AluInp.PREV_DELAY_5)
    for k in range(5, 8):
        dp[k].pass_through_alu()
    u.enable_output(OutSel.ALU_OUT, OutPath.WR0_LO)
    return u


class _AntOp:
    """Duck-types concourse.dve_ops.DveOp."""

    def __init__(self, name, uop, rd1, perf, spec):
        self.name = name
        self.spec = spec
        self.rd1 = rd1
        self.subdim = False
        self._uop = uop
        self._perf = perf

    def compile(self, ver):
        assert ver == "v3"
        if self._perf:
            return DveOpSpec(
                name=self.name,
                opcode=_dve_ops_mod.get_dve_sub_opcode(self.name),
                uops=[_marker1x(False)], uops_2x=[_marker1x(True)],
                uops_2x_2p=[self._uop], uops_4x=None,
                perf_max=2, rd1_en=self.rd1)
        return DveOpSpec(
            name=self.name, opcode=_dve_ops_mod.get_dve_sub_opcode(self.name),
            uops=[self._uop], rd1_en=self.rd1)


def _register(op):
    existing = next((o for o in _dve_ops_mod.OPS if o.name == op.name), None)
    if existing is not None:
        return existing
    _dve_ops_mod.OPS.append(op)
    row = _dve_ops_mod._CUSTOM_DVE_ROW_BASE + len(_dve_ops_mod.OPS) - 1
    assert row < 0x20, row
    _dve_ops_mod._SUB_OPCODE_FOR_NAME[op.name] = row
    _dve_ops_mod.CUSTOM_DVE_SPECS[op.name] = op.spec
    return op


_SPECA = Spec(body=Src0 + Src0 * _SC0 + Src0 * _SC1,
              reference=lambda in0, in1, s0, s1, imm2:
                  in0 + in0 * s0 + in0 * s1)
_SPECB = Spec(body=Src1 - (Src0 + Src0 * _SC1) * _SC0,
              reference=lambda in0, in1, s0, s1, imm2:
                  in1 - (in0 + in0 * s1) * s0)

_ops = {}


def _get_ops():
    if not _ops:
        _ops["A"] = _register(_AntOp("ZA2X_ANT", _opa_2x2p(), False, True,
                                     _SPECA))
        _ops["B"] = _register(_AntOp("SQB1X_ANT", _opb_1x(True), True, False,
                                     _SPECB))
        _ops["F"] = _register(_AntOp("GFB1X_ANT", _opb_1x(False), True, False,
                                     _SPECB))
    return _ops


def _fold2(t, col0, w):
    """[128, 2, w] AP over tile t: row0 at col0, row1 at col0 + w - 2
    (2-col overlap; port0's tail overwrites port1's glitched head)."""
    full = t[:, :]
    pstride = full.ap[0][0]
    return AP(full.tensor, full.offset + col0,
              [[pstride, 128], [w - 2, 2], [1, w]])


# --- kernel build -----------------------------------------------------------

def _build(steps: int, loop_k: int | None = None, x_early: int | None = None,
           x_late: int | None = None, r_steps: int | None = None):
    x_early = X_EARLY if x_early is None else x_early
    x_late = X_LATE if x_late is None else x_late
    r_steps = R_STEPS if r_steps is None else r_steps
    ops = _get_ops()
    assert x_early <= MAXX and (r_steps == 0 or x_late >= XLMIN)
    x_min = min(x_early, x_late if r_steps > 0 else x_early)
    pe_w = T - XLMIN            # Dhi/Dlo span (allocation bound)
    sqr_w = T - XLMIN + 4       # fp32r sq staging span; origin x_late-4
    nc = bacc.Bacc("TRN2", target_bir_lowering=False, debug=False)

    x_in = nc.dram_tensor("x", [ROWS, T], F32, kind="ExternalInput").ap()
    coef = nc.dram_tensor("coef", [ROWS, 8], F32, kind="ExternalInput").ap()
    wdiag = nc.dram_tensor("wdiag", [ROWS, 640], F32,
                           kind="ExternalInput").ap()
    out = nc.dram_tensor("out", [ROWS, T], F32, kind="ExternalOutput").ap()

    x_h = x_in.rearrange("(h p) t -> h p t", p=128)
    out_h = out.rearrange("(h p) t -> h p t", p=128)
    coef_h = coef.rearrange("(h p) c -> h p c", p=128)
    wdiag_h = wdiag.rearrange("(h p) c -> h p c", p=128)

    mult = mybir.AluOpType.mult
    add = mybir.AluOpType.add

    with tile.TileContext(nc) as tc:
        with tc.tile_pool(name="state", bufs=1) as pool, \
             tc.tile_pool(name="psum", bufs=8, space="PSUM") as pspool:
            neg_half = pool.tile([128, 1], F32, tag="nh", name="nh")
            nc.vector.memset(neg_half[:], -0.5)
            sqq, zab, dpb, cf, wd = [], [], [], [], []
            wir, wdr, Dhi, Dlo, sqr = [], [], [], [], []
            for h in range(HALVES):
                sqq.append([
                    pool.tile([128, PADC + T], F32, tag=f"sqa{h}",
                              name=f"sqa{h}"),
                    pool.tile([128, PADC + T], F32, tag=f"sqb{h}",
                              name=f"sqb{h}"),
                ])
                zab.append(pool.tile([128, PADC + MAXX], F32, tag=f"za{h}",
                                     name=f"za{h}"))
                dpb.append(pool.tile([128, PADC + MAXX], F32, tag=f"dp{h}",
                                     name=f"dp{h}"))
                cf.append(pool.tile([128, 8], F32, tag=f"cf{h}", name=f"cf{h}"))
                wd.append(pool.tile([128, 640], F32, tag=f"wd{h}",
                                    name=f"wd{h}"))
                wir.append(pool.tile([128, 128], F32R, tag=f"wir{h}",
                                     name=f"wir{h}"))
                Dhi.append(pool.tile([128, pe_w], F32R, tag=f"dh{h}",
                                     name=f"dh{h}"))
                Dlo.append(pool.tile([128, pe_w], F32R, tag=f"dl{h}",
                                     name=f"dl{h}"))
                if r_steps > 0:
                    wdr.append(pool.tile([128, 512], F32R, tag=f"wdr{h}",
                                         name=f"wdr{h}"))
                    sqr.append([
                        pool.tile([128, sqr_w], F32R, tag=f"sra{h}",
                                  name=f"sra{h}"),
                        pool.tile([128, sqr_w], F32R, tag=f"srb{h}",
                                  name=f"srb{h}"),
                    ])

            for h in range(HALVES):
                nc.sync.dma_start(out=cf[h][:], in_=coef_h[h])
                nc.sync.dma_start(out=wd[h][:], in_=wdiag_h[h])
                # x lands in sqB's data region (rewritten at step 0)
                nc.sync.dma_start(out=sqq[h][1][:, PADC:PADC + T], in_=x_h[h])
                nc.scalar.copy(wir[h][:], wd[h][:, 512:640])
                if r_steps > 0:
                    nc.scalar.copy(wdr[h][:], wd[h][:, 0:512])
                for p in range(2):
                    nc.vector.memset(sqq[h][p][:, 0:PADC], 0.25)
                nc.vector.memset(zab[h][:, 0:PADC], 0.0)
                xsrc = sqq[h][1][:, PADC:PADC + T]
                # D' = beta*x0 + (dconst - 0.5); pads = host dpad column
                nc.vector.tensor_scalar(
                    dpb[h][:, PADC:PADC + MAXX], xsrc[:, 0:MAXX],
                    cf[h][:, 4:5], cf[h][:, 5:6], mult, add)
                nc.vector.tensor_scalar(
                    dpb[h][:, 0:PADC], sqq[h][0][:, 0:PADC],
                    0.0, cf[h][:, 6:7], mult, add)
                # D true over the PE span -> Dhi/Dlo (fp32r split); sqA data
                # region used as init scratch (sq_0 overwrites it after)
                scr = sqq[h][0][:, PADC:PADC + pe_w]
                nc.vector.tensor_scalar(
                    scr, xsrc[:, XLMIN:T],
                    cf[h][:, 4:5], cf[h][:, 7:8], mult, add)
                nc.scalar.copy(Dhi[h][:], scr)
                nc.vector.tensor_tensor(
                    Dlo[h][:], scr, Dhi[h][:].bitcast(F32),
                    mybir.AluOpType.subtract)
                # sq_0 = (x - 0.5)^2
                nc.scalar.activation(
                    sqq[h][0][:, PADC:PADC + T], xsrc,
                    mybir.ActivationFunctionType.Square, bias=neg_half[:])

            def emit_step(s, final):
                use_r = r_steps > 0 and s >= steps - r_steps
                nxt_r = (r_steps > 0 and not final
                         and (s + 1) >= steps - r_steps)
                xs = x_late if use_r else x_early
                sq0 = x_late - 4          # sqr tile origin (main-sq col)
                for h in range(HALVES):
                    cur = sqq[h][s % 2]
                    nxt = sqq[h][(s + 1) % 2]
                    # opA: zA over cols [2, PADC+xs), folded 2x
                    L = PADC + xs - 2
                    W = (L + 2) // 2
                    bi = nc.vector._custom_dve(
                        ops["A"], out=_fold2(zab[h], 2, W),
                        in0=_fold2(cur, 2, W),
                        s0=cf[h][:, 0:1], s1=cf[h][:, 1:2])
                    bi.ins.perf_max = 2
                    # PE region: ps = sum -Cj sq[t-j] + D
                    blocks = []
                    c = xs
                    while c < T:
                        n = min(PE_BLOCK, T - c)
                        ps = pspool.tile([128, PE_BLOCK], F32, tag="ps",
                                         name=f"ps{s}_{h}_{c}")
                        for k in range(4):
                            if use_r:
                                # moving operand from the fp32r staging tile
                                nc.tensor.matmul(
                                    ps[:, :n],
                                    wdr[h][:, k * 128:(k + 1) * 128],
                                    sqr[h][s % 2][:, c - k - sq0:
                                                  c - k - sq0 + n],
                                    start=(k == 0), stop=False)
                            else:
                                nc.tensor.matmul(
                                    ps[:, :n],
                                    wd[h][:, k * 128:(k + 1) * 128],
                                    cur[:, PADC - k + c:PADC - k + c + n],
                                    start=(k == 0), stop=False)
                        rc = c - XLMIN
                        nc.tensor.matmul(ps[:, :n], wir[h][:],
                                         Dhi[h][:, rc:rc + n],
                                         start=False, stop=False)
                        nc.tensor.matmul(ps[:, :n], wir[h][:],
                                         Dlo[h][:, rc:rc + n],
                                         start=False, stop=True)
                        blocks.append((c, n, ps))
                        c += n
                    # opB: sq' (or final g') over cols [3, PADC+xs)
                    nc.vector._custom_dve(
                        ops["F" if final else "B"],
                        out=nxt[:, 3:PADC + xs],
                        in0=zab[h][:, 3:PADC + xs],
                        in1=dpb[h][:, 3:PADC + xs],
                        s0=cf[h][:, 2:3], s1=cf[h][:, 3:4],
                        imm2=0.5 if final else 0.0)
                    # ScalarE: PE region square from PSUM (or copy on final).
                    # When the NEXT step uses fp32r taps, the square goes to
                    # the fp32r staging tile instead of the main sq buffer,
                    # plus a round-copy of the opB-written strip
                    # [x_next-3, xs) so taps below x_next are covered.
                    for (c, n, ps) in blocks:
                        if final:
                            nc.scalar.copy(nxt[:, PADC + c:PADC + c + n],
                                           ps[:, :n])
                        elif nxt_r:
                            nc.scalar.activation(
                                sqr[h][(s + 1) % 2][:, c - sq0:c - sq0 + n],
                                ps[:, :n],
                                mybir.ActivationFunctionType.Square,
                                bias=neg_half[:])
                        else:
                            nc.scalar.activation(
                                nxt[:, PADC + c:PADC + c + n], ps[:, :n],
                                mybir.ActivationFunctionType.Square,
                                bias=neg_half[:])
                    if nxt_r:
                        # strip [x_late-3, xs): rounded copy of opB's output
                        nc.scalar.copy(
                            sqr[h][(s + 1) % 2][:, 1:1 + (xs - x_late + 3)],
                            nxt[:, PADC + x_late - 3:PADC + xs])

            if loop_k is not None:
                with tc.For_i(0, loop_k):
                    for s in range(steps):
                        emit_step(s, final=False)
            else:
                for s in range(steps):
                    emit_step(s, final=(s == steps - 1))

            fin = [sqq[h][steps % 2] for h in range(HALVES)]
            for h in range(HALVES):
                nc.vector.tensor_scalar(
                    fin[h][:, PADC:PADC + T], fin[h][:, PADC:PADC + T],
                    CLAMP, 1.0 - CLAMP,
                    mybir.AluOpType.max, mybir.AluOpType.min)
                nc.sync.dma_start(out=out_h[h], in_=fin[h][:, PADC:PADC + T])

    nc.compile()
    return nc


def get_nc(steps: int):
    if steps not in _compiled:
        _compiled[steps] = _build(steps)
    return _compiled[steps]


# --- host prep --------------------------------------------------------------

def _solve_za(C0, C1, C2, C3):
    """Per-channel real root of the zA-factorization cubic."""
    n = len(C0)
    mus = np.zeros(n); nus = np.zeros(n); bets = np.zeros(n)
    for c in range(n):
        a3, a2, a1, a0 = C3[c], C2[c], C1[c], C0[c]
        coeffs = [a3 ** 2, -2 * a2 * a3, a2 ** 2 + a1 * a3, a3 * a0 - a1 * a2]
        best = None
        for rt in np.roots(coeffs):
            if abs(rt.imag) > 1e-9 * max(1.0, abs(rt.real)):
                continue
            mu = rt.real
            b_ = a2 - a3 * mu
            if abs(b_) < 1e-9 or abs(mu) < 1e-7:
                continue
            nu = a0 / b_
            cond = abs(mu) + abs(nu) + abs(b_ / a3) + abs(nu / mu)
            if best is None or cond < best[0]:
                best = (cond, mu, nu, b_)
        assert best is not None, f"no usable root for channel {c}"
        _, mus[c], nus[c], bets[c] = best
    return mus, nus, bets


def _host_prep(drive, r, eps, beta, K_causal):
    """Per-core inputs: x (256,T), coef (256,8), wdiag (256,640)."""
    f32 = np.float32
    drive = np.asarray(drive, f32)
    r = np.asarray(r, np.float64)
    eps = np.asarray(eps, np.float64)
    beta = np.asarray(beta, np.float64)
    K = np.asarray(K_causal, np.float64)[:, 0, :]  # (C, 4)

    one_m_b = 1.0 - beta
    C0 = one_m_b * eps * r * K[:, 0]
    C1 = one_m_b * eps * r * K[:, 1]
    C2 = one_m_b * eps * r * K[:, 2]
    C3 = one_m_b * r * ((1.0 - eps) + eps * K[:, 3])
    dconst = 0.25 * (C0 + C1 + C2 + C3)
    mus, nus, bets = _solve_za(C0, C1, C2, C3)

    mu32 = mus.astype(f32)
    nom32 = (nus.astype(f32) / mus.astype(f32)).astype(f32)
    al32 = C3.astype(f32)
    boa32 = (bets.astype(f32) / C3.astype(f32)).astype(f32)
    # device-exact pad fixed point: D'pad = fl(P_pad) - 0.5
    q = f32(0.25)
    m1p = (mu32 * q).astype(f32)
    m2p = (nom32 * m1p).astype(f32)
    Zp = ((q + m1p).astype(f32) + m2p).astype(f32)
    mp = (al32 * Zp).astype(f32)
    wp = (boa32 * mp).astype(f32)
    Pp = (wp + mp).astype(f32)
    dpad = (Pp - f32(0.5)).astype(f32)

    in_maps = []
    idx = np.arange(128)
    for i in range(N_CORES):
        sl = slice(i * CPC, (i + 1) * CPC)
        xs = np.ascontiguousarray(
            drive[:, :, sl].transpose(0, 2, 1).reshape(ROWS, T), f32)
        cs = np.stack(
            [np.tile(mu32[sl], B), np.tile(nom32[sl], B),
             np.tile(al32[sl], B), np.tile(boa32[sl], B),
             np.tile(beta[sl].astype(f32), B),
             np.tile((dconst.astype(f32) - f32(0.5))[sl], B),
             np.tile(dpad[sl], B), np.tile(dconst.astype(f32)[sl], B)],
            axis=1).astype(f32)
        blocks = [-C3, -C2, -C1, -C0, np.ones(C)]
        wdg = np.zeros((ROWS, 640), f32)
        for k, arr in enumerate(blocks):
            rows = np.tile(arr.astype(f32)[sl], B)
            for h in range(HALVES):
                wdg[h * 128 + idx, k * 128 + idx] = rows[h * 128 + idx]
        in_maps.append({"x": xs, "coef": np.ascontiguousarray(cs),
                        "wdiag": wdg})
    return in_maps


def kernel(drive, r, eps, beta, K_causal, steps):
    steps = int(steps)
    nc = get_nc(steps)
    in_maps = _host_prep(drive, r, eps, beta, K_causal)
    res = run_bass_kernel_spmd(nc, in_maps, list(range(N_CORES)))
    parts = [
        res.results[i]["out"].reshape(B, CPC, T).transpose(0, 2, 1)
        for i in range(N_CORES)
    ]
    return np.ascontiguousarray(np.concatenate(parts, axis=2), np.float32)
